# revision 34
# baseline (speedup 1.0000x reference)
"""Block-quantized FP8 linear (KLinearFP8) on 8 trn2 NeuronCores.

y[m, n] = sum_k x_dq[m, k] * w_dq[n, k]
  x_dq: per-(row, 128-block) fp8e4m3fn-simulated quantization of x
  w_dq: weight (fp8 values held in fp32) * per-128x128-block scale

Sharding: column-parallel. weight/weight_scale_inv split along N across
8 cores, x replicated; each core computes y[:, c*2048:(c+1)*2048] as a
4096x4096x2048 bf16 GEMM (fp32 PSUM) -- ~874us of pure matmul at the
2.4GHz PE rate, which this kernel runs at ~94% density.

Host-side prep (layout/dtype/scale transforms; same arithmetic, same
single-rounding steps the chip would do):
  xq: x pre-quantized on the reference grid (xq = x/(2*s_x) in fp8,
      s_x = amax/448; factor-2 power-of-two rescale keeps values <=224
      < TRN e4m3 max 240 with identical rounding). 4x less x DMA.
  s2: 2*s_x scales laid out [m%128, m//128, kb] for per-partition use.
  wt: w_dq pre-dequantized to bf16, transposed [K, NSH]; k-slabs DMA
      straight into the K-on-partitions layout the PE streams.
  eye: 128x128 identity for PE-mode transposes.

On-chip per m-tile: one 512KB xq DMA, two DVE dequants (fp8*s2->bf16),
two XBAR-DMA transposes to K-on-partitions, 128 matmuls, per-chunk
drains (ACT copy + SWDGE y store) inline right after each chunk's stop
matmul so PSUM banks recycle ~38us before reuse.

Startup (the hard part -- lessons from ~15 traced iterations):
 - First two m-tiles run as ONE joint kb-major block: 8 MMs per k-slab
   across both tiles' 8 PSUM banks (~1.7us/slab) pacing consumption to
   the 16MB weight-stream arrival (~1.5us/slab), instead of racing
   ahead and stalling.
 - Weights ride the scalar HWDGE ring in groups (1,1,1,1,2,2,4...):
   only ~4 HWDGE DMAs are admitted at once and later DMAs chain on
   earlier completions, so small leading quanta keep the critical
   x-path transposes from parking behind multi-us weight transfers.
   SWDGE is NOT usable for weights: the framework serializes
   DMA_TRANSPOSE against in-flight SWDGE DMAs (deadlock guard).
 - Keeping dequant+transpose on-chip is deliberate: a host-prepared
   fully-dense variant (pure GEMM, zero gaps) reproducibly downclocked
   the PE 2.4->2.0GHz (power-state), costing ~160us. This shape holds
   2.4GHz.
"""

import numpy as np

M, K, N = 4096, 4096, 16384
NCORES = 8
NSH = N // NCORES
P = 128
KB = K // P
KH = KB // 2
MT = M // P
NB = NSH // P
CHW = 512
FP8_MAX = 448.0

_NC_CACHE = {}


def _build(M=M, K=K, NSH=NSH, debug=False):
    import concourse.bass as bass  # noqa: F401
    import concourse.mybir as mybir
    import concourse.tile as tile
    from concourse import bacc

    KB = K // P
    KH = KB // 2
    MT = M // P
    NB = NSH // P
    CHW = min(512, NSH)
    NCH = NSH // CHW
    NJOIN = min(2, MT)

    f32, bf16, f8 = mybir.dt.float32, mybir.dt.bfloat16, mybir.dt.float8e4

    nc = bacc.Bacc(None, target_bir_lowering=False, debug=debug)
    xq_d = nc.declare_dram_parameter("xq", [M, K], f8, isOutput=False)
    s2_d = nc.declare_dram_parameter("s2", [P, MT, KB], f32, isOutput=False)
    eye_d = nc.declare_dram_parameter("eye", [P, P], bf16, isOutput=False)
    wt_d = nc.declare_dram_parameter("wt", [K, NSH], bf16, isOutput=False)
    y_d = nc.declare_dram_parameter("y", [M, NSH], bf16, isOutput=True)

    with tile.TileContext(nc) as tc:
        with (
            tc.tile_pool(name="const", bufs=1) as const,
            tc.tile_pool(name="wt", bufs=1) as wtp,
            tc.tile_pool(name="xq8", bufs=3) as xq8,
            tc.tile_pool(name="xdqp", bufs=3) as xdqp,
            tc.tile_pool(name="xtp", bufs=6) as xtp,
            tc.tile_pool(name="ypool", bufs=4) as ypool,
            tc.tile_pool(name="psum", bufs=8, space="PSUM") as psum,
        ):
            s2all = const.tile([P, MT, KB], f32)
            nc.scalar.dma_start(s2all[:], s2_d[:])
            eye = const.tile([P, P], bf16)
            nc.scalar.dma_start(eye[:], eye_d[:])

            def x_load(mt):
                ms = slice(mt * P, (mt + 1) * P)
                xq = xq8.tile([P, KB, P], f8, name="xq", tag="xq")
                nc.scalar.dma_start(
                    xq[:], xq_d[ms, :].rearrange("m (kb x) -> m kb x", x=P)
                )
                return xq

            def x_deq(mt, xq, kh):
                kbs = slice(kh * KH, (kh + 1) * KH)
                xdq = xdqp.tile([P, KH, P], bf16, name="xdq", tag="xdq")
                nc.vector.tensor_tensor(
                    xdq[:], xq[:, kbs, :],
                    s2all[:, mt, kbs][:, :, None].to_broadcast((P, KH, P)),
                    mybir.AluOpType.mult,
                )
                return xdq

            def x_tr_dma(xdq):
                xT = xtp.tile([P, KH, P], bf16, name="xT", tag="xT")
                nc.sync.dma_start_transpose(
                    xT[:], xdq[:].rearrange("p a b -> p (a b)")
                )
                return xT

            def x_tr_pe(xdq):
                # PE-side 128x128 transposes: used for the one half
                # (mt1 h0) whose XBAR-DMA transpose sits behind weight
                # DMAs in the HWDGE completion chain and stalls the
                # joint block ~15us. PSUM staging cycles tag-"pt" slots
                # before the joint's accumulators claim them.
                xT = xtp.tile([P, KH, P], bf16, name="xT", tag="xT")
                for kb in range(KH):
                    st = psum.tile([P, CHW], bf16, name="tstage", tag="pt")
                    nc.tensor.transpose(st[:, 0:P], xdq[:, kb, :], eye[:])
                    nc.scalar.activation(
                        xT[:, kb, :], st[:, 0:P],
                        mybir.ActivationFunctionType.Copy,
                    )
                return xT

            def x_prep(mt):
                xq = x_load(mt)
                return [x_tr_dma(x_deq(mt, xq, kh)) for kh in range(2)]

            def drain_chunk(mt, c, pt):
                ms = slice(mt * P, (mt + 1) * P)
                yt = ypool.tile([P, CHW], bf16, name="yt", tag="yt")
                nc.scalar.activation(
                    yt[:], pt[:], mybir.ActivationFunctionType.Copy
                )
                nc.gpsimd.dma_start(y_d[ms, c * CHW:(c + 1) * CHW], yt[:])

            if NJOIN == 2:
                # custom prep for the joint tiles: h0 dequants first
                # (they gate the joint's first matmuls), mt1-h0
                # transposed on the PE.
                xq0, xq1 = x_load(0), x_load(1)
                d00 = x_deq(0, xq0, 0)
                d10 = x_deq(1, xq1, 0)
                d01 = x_deq(0, xq0, 1)
                d11 = x_deq(1, xq1, 1)
                # sync-queue order [T00, T10, T01, T11]: each transpose
                # chains behind earlier DMA completions, and T01/T11
                # aren't consumed until k-slab 16 (~25us later) -- so
                # mt1-h0 takes the second (early) slot.
                t00 = x_tr_dma(d00)
                t10 = x_tr_dma(d10)
                t01 = x_tr_dma(d01)
                t11 = x_tr_dma(d11)
                xT_bufs = {0: [t00, t01], 1: [t10, t11]}
            else:
                xT_bufs = {t: x_prep(t) for t in range(min(NJOIN, MT))}

            # Weights ride the scalar HWDGE ring (SWDGE is serialized
            # against DMA transposes by the framework's deadlock guard).
            # Early groups are tiny: HWDGE admits new DMAs in a global
            # completion chain, so the first transposes wait on whatever
            # weight group is in flight -- small quanta, short waits.
            if KB >= 8:
                gsizes = [2, 2] + [4] * ((KB - 4) // 4)
            else:
                gsizes = [KB]
            wGs = []
            k0 = 0
            for g, gw in enumerate(gsizes):
                wG = wtp.tile([P, gw, NB, P], bf16, name="wG", tag=f"wG{g}")
                nc.scalar.dma_start(
                    wG[:].rearrange("p a b c -> p a (b c)"),
                    wt_d[k0 * P:(k0 + gw) * P, :].rearrange(
                        "(a p) n -> p a n", p=P
                    ),
                )
                wGs += [(wG, j) for j in range(gw)]
                k0 += gw

            for t in range(NJOIN, min(NJOIN + 2, MT)):
                xT_bufs[t] = x_prep(t)

            def wv(kb, c):
                wG, j = wGs[kb]
                return wG[:, j, :, :].rearrange("p a b -> p (a b)")[
                    :, c * CHW:(c + 1) * CHW
                ]

            jpts = {
                t: [
                    psum.tile([P, CHW], f32, name=f"jpt{t}_{c}", tag="pt")
                    for c in range(NCH)
                ]
                for t in range(NJOIN)
            }
            # staggered kb-major schedule: mt0 runs STAG slabs solo
            # first (mt1's transpose lands ~8us after mt0's), then both
            # tiles interleave, then mt1 finishes solo on resident
            # slabs. Same 2*KB*NCH matmuls, consumption still paced to
            # weight arrival.
            STAG = min(8, KB // 2) if NJOIN == 2 else 0
            steps = (
                [(0, kb) for kb in range(STAG)]
                + [
                    (t, kb + (STAG if t == 0 else 0))
                    for kb in range(KB - STAG)
                    for t in range(NJOIN)
                ]
                + [(1, kb) for kb in range(KB - STAG, KB)]
                if NJOIN == 2
                else [(t, kb) for kb in range(KB) for t in range(NJOIN)]
            )
            for t, kb in steps:
                xh = xT_bufs[t][kb // KH]
                for c in range(NCH):
                    nc.tensor.matmul(
                        jpts[t][c][:], xh[:, kb % KH, :], wv(kb, c),
                        start=(kb == 0), stop=(kb == KB - 1),
                    )
            for t in range(NJOIN):
                xT_bufs.pop(t)
                for c in range(NCH):
                    drain_chunk(t, c, jpts[t][c])

            for mt in range(NJOIN, MT):
                xThalf = xT_bufs.pop(mt)
                if mt + 2 < MT:
                    xT_bufs[mt + 2] = x_prep(mt + 2)
                pts = [
                    psum.tile([P, CHW], f32, name=f"pt{c}", tag="pt")
                    for c in range(NCH)
                ]
                for kh in range(2):
                    for c in range(NCH):
                        for kb in range(KH):
                            nc.tensor.matmul(
                                pts[c][:],
                                xThalf[kh][:, kb, :],
                                wv(kh * KH + kb, c),
                                start=(kh == 0 and kb == 0),
                                stop=(kh == 1 and kb == KH - 1),
                            )
                        if kh == 1:
                            drain_chunk(mt, c, pts[c])

    nc.compile()
    return nc


def _host_quant_x(x):
    import ml_dtypes

    Mx, Kx = x.shape
    kb = Kx // P
    xb = x.reshape(Mx, kb, P)
    amax = np.abs(xb).max(axis=-1)
    s_x = (amax / np.float32(FP8_MAX)).astype(np.float32)
    s2 = s_x * np.float32(2.0)
    with np.errstate(divide="ignore", invalid="ignore"):
        xq = (xb / s2[:, :, None]).astype(ml_dtypes.float8_e4m3)
    xq = np.ascontiguousarray(xq.reshape(Mx, Kx))
    s2l = np.ascontiguousarray(s2.reshape(Mx // P, P, kb).transpose(1, 0, 2))
    return xq, s2l


def _core_inputs(xq, s2l, weight, ws, c, nsh=NSH, nb=NB):
    import ml_dtypes

    kb = weight.shape[1] // P
    wsl = weight[c * nsh:(c + 1) * nsh]
    scale = ws[c * nb:(c + 1) * nb]
    wdq = (
        wsl.reshape(nb, P, kb, P) * scale[:, None, :, None].astype(np.float32)
    ).reshape(nsh, weight.shape[1])
    wt = np.ascontiguousarray(wdq.T).astype(ml_dtypes.bfloat16)
    eye = np.eye(P, dtype=ml_dtypes.bfloat16)
    return {"xq": xq, "s2": s2l, "eye": eye, "wt": wt}


def kernel(x, weight, weight_scale_inv):
    from concourse.bass_utils import run_bass_kernel_spmd

    if "nc" not in _NC_CACHE:
        _NC_CACHE["nc"] = _build()
    nc = _NC_CACHE["nc"]

    x = np.ascontiguousarray(np.asarray(x, dtype=np.float32))
    weight = np.asarray(weight, dtype=np.float32)
    ws = np.asarray(weight_scale_inv, dtype=np.float32)

    xq, s2l = _host_quant_x(x)
    in_maps = [_core_inputs(xq, s2l, weight, ws, c) for c in range(NCORES)]
    res = run_bass_kernel_spmd(nc, in_maps, list(range(NCORES)))
    y = np.concatenate(
        [np.asarray(res.results[c]["y"]) for c in range(NCORES)], axis=1
    )
    return y.astype(np.float32, copy=False)


# revision 36
# speedup vs baseline: 1.1933x; 1.1933x over previous
"""Block-quantized FP8 linear (KLinearFP8) on 8 trn2 NeuronCores.

y[m, n] = sum_k x_dq[m, k] * w_dq[n, k]
  x_dq: per-(row, 128-block) fp8e4m3fn-simulated quantization of x
  w_dq: weight (fp8 values held in fp32) * per-128x128-block scale

Sharding: column-parallel. weight/weight_scale_inv split along N across
8 cores, x replicated; each core computes y[:, c*2048:(c+1)*2048] as a
4096x4096x2048 bf16 GEMM (fp32 PSUM) -- ~874us of pure matmul at the
2.4GHz PE rate, which this kernel runs at ~94% density.

Host-side prep (layout/dtype/scale transforms; same arithmetic, same
single-rounding steps the chip would do):
  xq: x pre-quantized on the reference grid (xq = x/(2*s_x) in fp8,
      s_x = amax/448; factor-2 power-of-two rescale keeps values <=224
      < TRN e4m3 max 240 with identical rounding). 4x less x DMA.
  s2: 2*s_x scales laid out [m%128, m//128, kb] for per-partition use.
  wt: w_dq pre-dequantized to bf16, transposed [K, NSH]; k-slabs DMA
      straight into the K-on-partitions layout the PE streams.
  eye: 128x128 identity for PE-mode transposes.

On-chip per m-tile: one 512KB xq DMA, two DVE dequants (fp8*s2->bf16),
two XBAR-DMA transposes to K-on-partitions, 128 matmuls, per-chunk
drains (ACT copy + SWDGE y store) inline right after each chunk's stop
matmul so PSUM banks recycle ~38us before reuse.

Startup (the hard part -- lessons from ~15 traced iterations):
 - First two m-tiles run as ONE joint kb-major block: 8 MMs per k-slab
   across both tiles' 8 PSUM banks (~1.7us/slab) pacing consumption to
   the 16MB weight-stream arrival (~1.5us/slab), instead of racing
   ahead and stalling.
 - Weights ride the scalar HWDGE ring in groups (1,1,1,1,2,2,4...):
   only ~4 HWDGE DMAs are admitted at once and later DMAs chain on
   earlier completions, so small leading quanta keep the critical
   x-path transposes from parking behind multi-us weight transfers.
   SWDGE is NOT usable for weights: the framework serializes
   DMA_TRANSPOSE against in-flight SWDGE DMAs (deadlock guard).
 - Keeping dequant+transpose on-chip is deliberate: a host-prepared
   fully-dense variant (pure GEMM, zero gaps) reproducibly downclocked
   the PE 2.4->2.0GHz (power-state), costing ~160us. This shape holds
   2.4GHz.
"""

import numpy as np

M, K, N = 4096, 4096, 16384
NCORES = 8
NSH = N // NCORES
P = 128
KB = K // P
KH = KB // 2
MT = M // P
NB = NSH // P
CHW = 512
FP8_MAX = 448.0

_NC_CACHE = {}


def _build(M=M, K=K, NSH=NSH, debug=False):
    import concourse.bass as bass  # noqa: F401
    import concourse.mybir as mybir
    import concourse.tile as tile
    from concourse import bacc

    KB = K // P
    KH = KB // 2
    MT = M // P
    NB = NSH // P
    CHW = min(512, NSH)
    NCH = NSH // CHW
    NJOIN = min(2, MT)

    f32, bf16, f8 = mybir.dt.float32, mybir.dt.bfloat16, mybir.dt.float8e4

    nc = bacc.Bacc(None, target_bir_lowering=False, debug=debug)
    xq_d = nc.declare_dram_parameter("xq", [M, K], f8, isOutput=False)
    s2_d = nc.declare_dram_parameter("s2", [P, MT, KB], f32, isOutput=False)
    eye_d = nc.declare_dram_parameter("eye", [P, P], bf16, isOutput=False)
    wt_d = nc.declare_dram_parameter("wt", [K, NSH], bf16, isOutput=False)
    y_d = nc.declare_dram_parameter("y", [M, NSH], bf16, isOutput=True)

    with tile.TileContext(nc) as tc:
        with (
            tc.tile_pool(name="const", bufs=1) as const,
            tc.tile_pool(name="wt", bufs=1) as wtp,
            tc.tile_pool(name="xq8", bufs=3) as xq8,
            tc.tile_pool(name="xdqp", bufs=3) as xdqp,
            tc.tile_pool(name="xtp", bufs=6) as xtp,
            tc.tile_pool(name="ypool", bufs=4) as ypool,
            tc.tile_pool(name="psum", bufs=8, space="PSUM") as psum,
        ):
            s2all = const.tile([P, MT, KB], f32)
            nc.scalar.dma_start(s2all[:], s2_d[:])
            eye = const.tile([P, P], bf16)
            nc.scalar.dma_start(eye[:], eye_d[:])

            def x_load(mt):
                ms = slice(mt * P, (mt + 1) * P)
                xq = xq8.tile([P, KB, P], f8, name="xq", tag="xq")
                nc.scalar.dma_start(
                    xq[:], xq_d[ms, :].rearrange("m (kb x) -> m kb x", x=P)
                )
                return xq

            def x_deq(mt, xq, kh):
                kbs = slice(kh * KH, (kh + 1) * KH)
                xdq = xdqp.tile([P, KH, P], bf16, name="xdq", tag="xdq")
                nc.vector.tensor_tensor(
                    xdq[:], xq[:, kbs, :],
                    s2all[:, mt, kbs][:, :, None].to_broadcast((P, KH, P)),
                    mybir.AluOpType.mult,
                )
                return xdq

            def x_tr_dma(xdq):
                xT = xtp.tile([P, KH, P], bf16, name="xT", tag="xT")
                nc.sync.dma_start_transpose(
                    xT[:], xdq[:].rearrange("p a b -> p (a b)")
                )
                return xT

            def x_tr_pe(xdq):
                # PE-side 128x128 transposes: used for the one half
                # (mt1 h0) whose XBAR-DMA transpose sits behind weight
                # DMAs in the HWDGE completion chain and stalls the
                # joint block ~15us. PSUM staging cycles tag-"pt" slots
                # before the joint's accumulators claim them.
                xT = xtp.tile([P, KH, P], bf16, name="xT", tag="xT")
                for kb in range(KH):
                    st = psum.tile([P, CHW], bf16, name="tstage", tag="pt")
                    nc.tensor.transpose(st[:, 0:P], xdq[:, kb, :], eye[:])
                    nc.scalar.activation(
                        xT[:, kb, :], st[:, 0:P],
                        mybir.ActivationFunctionType.Copy,
                    )
                return xT

            def x_prep(mt):
                xq = x_load(mt)
                return [x_tr_dma(x_deq(mt, xq, kh)) for kh in range(2)]

            def drain_chunk(mt, c, pt):
                ms = slice(mt * P, (mt + 1) * P)
                yt = ypool.tile([P, CHW], bf16, name="yt", tag="yt")
                nc.scalar.activation(
                    yt[:], pt[:], mybir.ActivationFunctionType.Copy
                )
                nc.gpsimd.dma_start(y_d[ms, c * CHW:(c + 1) * CHW], yt[:])

            # HAM warm-up: dummy PE transposes of eye into a never-read
            # PSUM tile keep the PE activity monitor busy from ~10us
            # (PE is otherwise idle until the first MM at ~25us), so
            # the clock gate opens to 8/8 before the first real matmuls
            # instead of ramping at 1.2GHz. The staging slot is
            # recycled by a later accumulator whose start=True matmul
            # clears it -- nothing ever reads the warm-up output.
            warm = psum.tile([P, CHW], bf16, name="warm", tag="pt")
            for _ in range(24):
                nc.tensor.transpose(warm[:, 0:P], eye[:], eye[:])

            if NJOIN == 2:
                # custom prep for the joint tiles: h0 dequants first
                # (they gate the joint's first matmuls), mt1-h0
                # transposed on the PE.
                xq0, xq1 = x_load(0), x_load(1)
                d00 = x_deq(0, xq0, 0)
                d10 = x_deq(1, xq1, 0)
                d01 = x_deq(0, xq0, 1)
                d11 = x_deq(1, xq1, 1)
                # sync-queue order [T00, T10, T01, T11]: each transpose
                # chains behind earlier DMA completions, and T01/T11
                # aren't consumed until k-slab 16 (~25us later) -- so
                # mt1-h0 takes the second (early) slot.
                t00 = x_tr_dma(d00)
                t10 = x_tr_dma(d10)
                t01 = x_tr_dma(d01)
                t11 = x_tr_dma(d11)
                xT_bufs = {0: [t00, t01], 1: [t10, t11]}
            else:
                xT_bufs = {t: x_prep(t) for t in range(min(NJOIN, MT))}

            # Weights ride the scalar HWDGE ring (SWDGE is serialized
            # against DMA transposes by the framework's deadlock guard).
            # Early groups are tiny: HWDGE admits new DMAs in a global
            # completion chain, so the first transposes wait on whatever
            # weight group is in flight -- small quanta, short waits.
            if KB >= 8:
                gsizes = [2, 2] + [4] * ((KB - 4) // 4)
            else:
                gsizes = [KB]
            wGs = []
            k0 = 0
            for g, gw in enumerate(gsizes):
                wG = wtp.tile([P, gw, NB, P], bf16, name="wG", tag=f"wG{g}")
                nc.scalar.dma_start(
                    wG[:].rearrange("p a b c -> p a (b c)"),
                    wt_d[k0 * P:(k0 + gw) * P, :].rearrange(
                        "(a p) n -> p a n", p=P
                    ),
                )
                wGs += [(wG, j) for j in range(gw)]
                k0 += gw

            for t in range(NJOIN, min(NJOIN + 2, MT)):
                xT_bufs[t] = x_prep(t)

            def wv(kb, c):
                wG, j = wGs[kb]
                return wG[:, j, :, :].rearrange("p a b -> p (a b)")[
                    :, c * CHW:(c + 1) * CHW
                ]

            jpts = {
                t: [
                    psum.tile([P, CHW], f32, name=f"jpt{t}_{c}", tag="pt")
                    for c in range(NCH)
                ]
                for t in range(NJOIN)
            }
            # staggered kb-major schedule: mt0 runs STAG slabs solo
            # first (mt1's transpose lands ~8us after mt0's), then both
            # tiles interleave, then mt1 finishes solo on resident
            # slabs. Same 2*KB*NCH matmuls, consumption still paced to
            # weight arrival.
            STAG = min(8, KB // 2) if NJOIN == 2 else 0
            steps = (
                [(0, kb) for kb in range(STAG)]
                + [
                    (t, kb + (STAG if t == 0 else 0))
                    for kb in range(KB - STAG)
                    for t in range(NJOIN)
                ]
                + [(1, kb) for kb in range(KB - STAG, KB)]
                if NJOIN == 2
                else [(t, kb) for kb in range(KB) for t in range(NJOIN)]
            )
            for t, kb in steps:
                xh = xT_bufs[t][kb // KH]
                for c in range(NCH):
                    nc.tensor.matmul(
                        jpts[t][c][:], xh[:, kb % KH, :], wv(kb, c),
                        start=(kb == 0), stop=(kb == KB - 1),
                    )
            for t in range(NJOIN):
                xT_bufs.pop(t)
                for c in range(NCH):
                    drain_chunk(t, c, jpts[t][c])

            for mt in range(NJOIN, MT):
                xThalf = xT_bufs.pop(mt)
                if mt + 2 < MT:
                    xT_bufs[mt + 2] = x_prep(mt + 2)
                pts = [
                    psum.tile([P, CHW], f32, name=f"pt{c}", tag="pt")
                    for c in range(NCH)
                ]
                for kh in range(2):
                    for c in range(NCH):
                        for kb in range(KH):
                            nc.tensor.matmul(
                                pts[c][:],
                                xThalf[kh][:, kb, :],
                                wv(kh * KH + kb, c),
                                start=(kh == 0 and kb == 0),
                                stop=(kh == 1 and kb == KH - 1),
                            )
                        if kh == 1:
                            drain_chunk(mt, c, pts[c])

    nc.compile()
    return nc


def _host_quant_x(x):
    import ml_dtypes

    Mx, Kx = x.shape
    kb = Kx // P
    xb = x.reshape(Mx, kb, P)
    amax = np.abs(xb).max(axis=-1)
    s_x = (amax / np.float32(FP8_MAX)).astype(np.float32)
    s2 = s_x * np.float32(2.0)
    with np.errstate(divide="ignore", invalid="ignore"):
        xq = (xb / s2[:, :, None]).astype(ml_dtypes.float8_e4m3)
    xq = np.ascontiguousarray(xq.reshape(Mx, Kx))
    s2l = np.ascontiguousarray(s2.reshape(Mx // P, P, kb).transpose(1, 0, 2))
    return xq, s2l


def _core_inputs(xq, s2l, weight, ws, c, nsh=NSH, nb=NB):
    import ml_dtypes

    kb = weight.shape[1] // P
    wsl = weight[c * nsh:(c + 1) * nsh]
    scale = ws[c * nb:(c + 1) * nb]
    wdq = (
        wsl.reshape(nb, P, kb, P) * scale[:, None, :, None].astype(np.float32)
    ).reshape(nsh, weight.shape[1])
    wt = np.ascontiguousarray(wdq.T).astype(ml_dtypes.bfloat16)
    eye = np.eye(P, dtype=ml_dtypes.bfloat16)
    return {"xq": xq, "s2": s2l, "eye": eye, "wt": wt}


def kernel(x, weight, weight_scale_inv):
    from concourse.bass_utils import run_bass_kernel_spmd

    if "nc" not in _NC_CACHE:
        _NC_CACHE["nc"] = _build()
    nc = _NC_CACHE["nc"]

    x = np.ascontiguousarray(np.asarray(x, dtype=np.float32))
    weight = np.asarray(weight, dtype=np.float32)
    ws = np.asarray(weight_scale_inv, dtype=np.float32)

    xq, s2l = _host_quant_x(x)
    in_maps = [_core_inputs(xq, s2l, weight, ws, c) for c in range(NCORES)]
    res = run_bass_kernel_spmd(nc, in_maps, list(range(NCORES)))
    y = np.concatenate(
        [np.asarray(res.results[c]["y"]) for c in range(NCORES)], axis=1
    )
    return y.astype(np.float32, copy=False)
